# revision 10
# baseline (speedup 1.0000x reference)
"""Trainium2 Bass kernel for a 4-head spatial MultiHeadAttention block.

Reference computation (per batch n):
    q/k/v = 1x1-conv projections of x (C=256 channels, S=48*48=2304 positions)
    per head (4 heads, d=64): attn = softmax(q^T k / 8), out = attn @ v
    out = Wo @ concat(heads) + bo + x   (residual)

Sharding across 8 NeuronCores: core c handles batch n = c//2 and head-pair
hp = c%2 (output channels [hp*128, hp*128+128) of the QKV projections, i.e.
heads {2*hp, 2*hp+1}).  Each core computes a partial output
Wo[:, ch] @ attn_ch (256 x 2304); the host sums the two partials per batch
and adds bo + residual x.

Per-core kernel layout choices (v2):
  - All inputs pre-packed on the host partition-major so every DMA moves
    large contiguous per-partition rows (x lands in ~4us in 3 transfers).
  - DMA issues are spread across engine queues (sync/gpsimd/scalar/vector);
    SBUF memsets run on the otherwise-idle GpSimd engine.
  - ScalarE does ONLY exp (the ~87us serial floor); every bias add / copy
    runs on VectorE.  A 1-element dummy Exp pins the ACT table set early.
  - Q stored (d, s); K zero-padded per head (kz0/kz1) so every scores
    matmul contracts the full 128 partitions with one PE config.
  - VT produced transposed by the projection matmul with constant-1 columns
    so attn@V also yields softmax row-sums (M=65); VT/K-tail/Q-tail
    projections are emitted in large batched PSUM allocations woven into
    the first attention unit.
  - scoresT(t,s): 3 t-tiles per 3-bank PSUM group; exp (ScalarE) runs on
    1536-wide batches out of PSUM; attn@V of batch g is emitted after the
    scores of batch g+1 (software pipeline).
  - normalization: reciprocal_approx_fast on the (1,sw) row-sum row, then
    gpsimd.partition_broadcast to 64 lanes (no DMA bounce), DVE multiply.
  - Wo projection per finished chunk with a paired single PSUM alloc;
    bf16 output staged in SBUF and DMA'd out at chunk-pair boundaries.
All matmul operands are bf16; accumulation and softmax math are fp32.
"""

import numpy as np

import concourse.bass as bass
import concourse.mybir as mybir
import concourse.tile as tile
from concourse import bacc
from concourse.bass_utils import run_bass_kernel_spmd

C = 256          # channels
S = 2304         # spatial positions (48*48)
HD = 64          # head dim
P = 128          # partitions
TT = S // P      # 18 t-tiles of 128
GRP = 3          # t-tiles per exp batch (3 PSUM banks)
NG = TT // GRP   # 6 exp batches per unit
SCALE = 0.125    # 1/sqrt(HD)
F32 = mybir.dt.float32
BF16 = mybir.dt.bfloat16

S_CHUNKS = [(0, 512), (512, 512), (1024, 512), (1536, 512), (2048, 256)]
CHUNK_ORDER = [4, 0, 1, 2, 3]  # small chunk first: its PSUM groups have
                                # 768 spare floats that carry the weave
# wqkv slot indices: [wk_a0, wk_a1, wq_a0, wq_a1, wv_a0, wv_a1]
WK0, WK1, WQ0, WQ1, WV0, WV1 = range(6)


def _body(tc):
    nc = tc.nc
    t_xx = nc.dram_tensor("xx", [P, 2, S], BF16, kind="ExternalInput").ap()
    t_wqkv = nc.dram_tensor("wqkv", [P, 6, P], BF16, kind="ExternalInput").ap()
    t_wot = nc.dram_tensor("wot", [P, C], BF16, kind="ExternalInput").ap()
    t_wotb = nc.dram_tensor("wotb", [HD, C], BF16, kind="ExternalInput").ap()
    t_bq = nc.dram_tensor("bq", [P, 1], F32, kind="ExternalInput").ap()
    t_bk = nc.dram_tensor("bk", [P, 1], F32, kind="ExternalInput").ap()
    t_bv = nc.dram_tensor("bv", [P, P], F32, kind="ExternalInput").ap()
    t_out = nc.dram_tensor("out", [2, P, S], BF16, kind="ExternalOutput").ap()

    singles = tc.alloc_tile_pool(name="singles", bufs=1)
    xx = singles.tile([P, 2, S], BF16)
    q_sb = singles.tile([P, S], BF16)
    kz0 = singles.tile([P, S], BF16)          # head A rows 0-63, zeros 64-127
    kz1 = singles.tile([P, S], BF16)          # zeros 0-63, head B rows 64-127
    vt_sb = singles.tile([P, TT, 130], BF16)  # per tt: [dA(64) | 1 | dB(64) | 1]
    wqkv_sb = singles.tile([P, 6, P], BF16)
    wot_sb = singles.tile([P, C], BF16)
    wotb_sb = singles.tile([HD, C], BF16)
    attn_full = singles.tile([P, S], BF16)
    ob = singles.tile([P, 2, S], BF16)        # output staging [p, half, s]
    bq_sb = singles.tile([P, 1], F32)
    bk_sb = singles.tile([P, 1], F32)
    bv_bc = singles.tile([P, P], F32)
    scr = singles.tile([1, 1], F32)

    # ---- input DMAs: wk slots first (gate the first matmul), then x as one
    # large-descriptor transfer; late-needed weights issued last ----
    nc.gpsimd.dma_start(out=wqkv_sb[:, 0:2, :], in_=t_wqkv[:, 0:2, :])
    nc.scalar.dma_start(out=bk_sb, in_=t_bk)
    nc.scalar.dma_start(out=bq_sb, in_=t_bq)
    nc.gpsimd.dma_start(out=wqkv_sb[:, 2:6, :], in_=t_wqkv[:, 2:6, :])
    nc.sync.dma_start(out=xx, in_=t_xx)
    # pin the exp table set now; input is a self-zeroed scratch (no DMA dep)
    nc.scalar.memzero(scr)
    nc.scalar.activation(scr, scr, mybir.ActivationFunctionType.Exp)
    nc.gpsimd.dma_start(out=bv_bc, in_=t_bv)
    nc.scalar.dma_start(out=wot_sb, in_=t_wot)
    nc.scalar.dma_start(out=wotb_sb, in_=t_wotb)
    # dead K halves + VT ones-columns; GpSimd is idle at startup
    nc.gpsimd.memset(kz0[HD:P, :], 0.0)
    nc.gpsimd.memset(kz1[0:HD, :], 0.0)
    nc.gpsimd.memset(vt_sb[:, :, HD:HD + 1], 1.0)
    nc.gpsimd.memset(vt_sb[:, :, 129:130], 1.0)

    ps = tc.alloc_tile_pool(name="ps", bufs=2, space="PSUM")
    ex_pool = tc.alloc_tile_pool(name="ex_sb", bufs=4)
    nrm = tc.alloc_tile_pool(name="nrm", bufs=2)

    def kq_piece(psn, kind, s0, sw):
        # K or Q projection for s range [s0, s0+sw) into the given PSUM region
        w0, w1 = (WK0, WK1) if kind == "k" else (WQ0, WQ1)
        pw = psn[:, :sw]
        nc.tensor.matmul(pw, wqkv_sb[:, w0, :], xx[:, 0, s0:s0 + sw],
                         start=True, stop=False)
        nc.tensor.matmul(pw, wqkv_sb[:, w1, :], xx[:, 1, s0:s0 + sw],
                         start=False, stop=True)
        if kind == "k":
            nc.vector.tensor_scalar_add(kz0[0:HD, s0:s0 + sw], pw[0:HD, :],
                                        bk_sb[0:HD, :])
            nc.vector.tensor_scalar_add(kz1[HD:P, s0:s0 + sw], pw[HD:P, :],
                                        bk_sb[HD:P, :])
        else:
            nc.vector.tensor_scalar_add(q_sb[:, s0:s0 + sw], pw, bq_sb)

    def vt_piece(psn, base, n):
        # n consecutive VT t-tiles into the given PSUM region, grouped adds
        ps3 = psn[:, :n * P].rearrange("p (n d) -> p n d", d=P)
        for j in range(n):
            tt = base + j
            nc.tensor.matmul(ps3[:, j, :], xx[:, 0, tt * P:(tt + 1) * P],
                             wqkv_sb[:, WV0, :], start=True, stop=False)
            nc.tensor.matmul(ps3[:, j, :], xx[:, 1, tt * P:(tt + 1) * P],
                             wqkv_sb[:, WV1, :], start=False, stop=True)
        # vt cols per tt: [dA 0:64 | one | dB 65:129 | one]; write both halves
        va = vt_sb[:, base:base + n, 0:HD]
        vb = vt_sb[:, base:base + n, HD + 1:129]
        pa = bass.AP(tensor=ps3.tensor, offset=ps3.offset,
                     ap=[ps3.ap[0], ps3.ap[1], [ps3.ap[2][0], HD]])
        pb_src = ps3[:, :, HD:P]
        bva = bass.AP(tensor=bv_bc.tensor, offset=bv_bc.offset,
                      ap=[bv_bc.ap[0], [0, n], [bv_bc.ap[1][0], HD]])
        bvb_base = bv_bc[:, HD:P]
        bvb = bass.AP(tensor=bvb_base.tensor, offset=bvb_base.offset,
                      ap=[bvb_base.ap[0], [0, n], bvb_base.ap[1]])
        nc.vector.tensor_add(va, pa, bva)
        nc.vector.tensor_add(vb, pb_src, bvb)

    def emit_weave(pieces, spare):
        off = 0
        for piece in pieces:
            if piece[0] == "vt":
                _, base, n = piece
                vt_piece(spare[:, off:off + n * P], base, n)
                off += n * P
            else:
                kind, s0, sw = piece
                kq_piece(spare[:, off:off + sw], kind, s0, sw)
                off += sw

    def emit_av(pend):
        ex, g, ot, h, sw = pend
        for j in range(GRP):
            tt = g * GRP + j
            nc.tensor.matmul(ot, vt_sb[:, tt, h * 65:(h + 1) * 65],
                             ex[:, j * sw:(j + 1) * sw],
                             start=(tt == 0), stop=(tt == TT - 1))

    def wo_chunk(ci, a1=None):
        s0, sw = S_CHUNKS[ci]
        psn = ps.tile([P, GRP * 512], F32, tag="sc", name="wops")
        for half in range(2):
            pw = psn[:, half * 512:half * 512 + sw]
            cs = slice(half * P, (half + 1) * P)
            if a1 is None:
                nc.tensor.matmul(pw, wot_sb[:, cs], attn_full[:, s0:s0 + sw],
                                 start=True, stop=True)
            else:
                nc.tensor.matmul(pw, wot_sb[0:HD, cs],
                                 attn_full[0:HD, s0:s0 + sw],
                                 start=True, stop=False)
                nc.tensor.matmul(pw, wotb_sb[:, cs], a1,
                                 start=False, stop=True)
            nc.vector.tensor_copy(ob[:, half, s0:s0 + sw], pw)

    def out_dma(s0, sw):
        for half in range(2):
            eng = nc.sync if half == 0 else nc.gpsimd
            eng.dma_start(out=t_out[half, :, s0:s0 + sw],
                          in_=ob[:, half, s0:s0 + sw])

    def emit_norm(ot, h, s0, sw, last=False):
        comb = nrm.tile([65, 512], F32, tag="comb", name="comb")[:, :sw]
        nc.vector.tensor_copy(comb, ot)
        # row-sum lives on partition 64; recip/broadcast need base partition 0
        rs0 = nrm.tile([1, 512], F32, tag="rs0", name="rs0")[:, :sw]
        nc.sync.dma_start(out=rs0, in_=comb[HD:HD + 1, :])
        rinv = nrm.tile([1, 512], F32, tag="rinv", name="rinv")[:, :sw]
        nc.vector.reciprocal_approx_fast(rinv, rs0)
        rb = nrm.tile([HD, 512], F32, tag="rb", name="rb")[:, :sw]
        nc.gpsimd.partition_broadcast(rb, rinv)
        if h == 0:
            nc.vector.tensor_mul(attn_full[0:HD, s0:s0 + sw], comb[0:HD, :], rb)
            return None
        a1 = nrm.tile([HD, 512], BF16, tag="a1", name="a1")[:, :sw]
        nc.vector.tensor_mul(a1, comb[0:HD, :], rb)
        if last:
            return a1
        nc.sync.dma_start(out=attn_full[HD:P, s0:s0 + sw], in_=a1)
        return None

    # ---- prologue compute: K chunks 0-1 + the small chunk's Q slice ----
    kq_piece(ps.tile([P, GRP * 512], F32, tag="sc", name="kps"), "k", 0, 512)
    kq_piece(ps.tile([P, GRP * 512], F32, tag="sc", name="qps"), "q", 2048, 256)
    kq_piece(ps.tile([P, GRP * 512], F32, tag="sc", name="kps"), "k", 512, 512)

    # weave: projection pieces riding in the 768-float spare space of the
    # small chunk's PSUM groups (no extra rotations).  The spare is a 256-
    # float slot (tail of bank 1) then a 512-float slot (bank 2); pieces
    # are sized so no matmul output crosses a 512-float bank boundary.
    WEAVE = {
        (1, 0): [("k", 1024, 256), ("k", 1280, 512)],
        (1, 1): [("vt", 0, 2), ("vt", 2, 4)],
        (1, 2): [("k", 1792, 256), ("k", 2048, 256), ("q", 0, 256)],
        (1, 3): [("vt", 6, 2), ("vt", 8, 4)],
        (1, 4): [("q", 256, 256), ("q", 512, 512)],
        (1, 5): [("vt", 12, 2), ("vt", 14, 4)],
        (2, 0): [("q", 1024, 256), ("q", 1280, 512)],
        (2, 1): [("q", 1792, 256)],
    }

    # ---- attention: software-pipelined across all (s-chunk, head) units ----
    pend = None       # (ex, g, ot, h, sw): exp batch whose attn@V is pending
    pend_norm = None  # (ot, h, s0, sw, ci): unit awaiting normalization
    unit = 0
    for ci in CHUNK_ORDER:
        s0, sw = S_CHUNKS[ci]
        for h in range(2):
            unit += 1
            kz = kz0 if h == 0 else kz1
            ot = ps.tile([65, 512], F32, tag="ot", name="ot")[:, :sw]
            for g in range(NG):
                scf = ps.tile([P, GRP * 512], F32, tag="sc", name="sc")
                sc = scf[:, :GRP * sw]
                for j in range(GRP):
                    tt = g * GRP + j
                    nc.tensor.matmul(sc[:, j * sw:(j + 1) * sw],
                                     kz[:, tt * P:(tt + 1) * P],
                                     q_sb[:, s0:s0 + sw],
                                     start=True, stop=True)
                pieces = WEAVE.get((unit, g))
                if pieces:
                    emit_weave(pieces, scf[:, GRP * sw:])
                if pend is not None:
                    emit_av(pend)
                    if pend[1] == NG - 1:  # last batch of its unit
                        emit_norm(*pend_norm[:4])
                ex = ex_pool.tile([P, GRP * 512], BF16, tag="ex", name="ex")[:, :GRP * sw]
                nc.scalar.activation(ex, sc, mybir.ActivationFunctionType.Exp,
                                     scale=SCALE)
                pend = (ex, g, ot, h, sw)
                if g == NG - 1:
                    pend_norm = (ot, h, s0, sw, ci)
    emit_av(pend)
    last_a1 = emit_norm(*pend_norm[:4], last=True)
    # Wo + output entirely after the pipeline; the last chunk's head-B Wo
    # contracts per-head against a1 directly (skips the a1->attn_full DMA)
    wo_chunk(4)
    wo_chunk(0)
    wo_chunk(1)
    out_dma(0, 1024)
    wo_chunk(2)
    out_dma(1024, 512)
    wo_chunk(3, last_a1)
    out_dma(1536, 768)

    nrm.release()
    ex_pool.release()
    ps.release()
    singles.release()


_NC_CACHE = {}


def build_nc():
    if "nc" not in _NC_CACHE:
        nc = bacc.Bacc("TRN2", target_bir_lowering=False, debug=False, num_devices=8)
        with tile.TileContext(nc) as tc:
            _body(tc)
        nc.compile()
        _NC_CACHE["nc"] = nc
    return _NC_CACHE["nc"]


def make_in_maps(x, Wq, bq, Wk, bk, Wv, bv, Wo, bo):
    import ml_dtypes
    bf16 = ml_dtypes.bfloat16
    N = x.shape[0]
    # (N, C, S) -> per batch (P, 2, S): partition p holds rows p and p+128
    xf = np.asarray(x, np.float32).reshape(N, C, S).reshape(N, 2, P, S)
    xf = np.ascontiguousarray(xf.transpose(0, 2, 1, 3).astype(bf16))
    in_maps = []
    for c in range(8):
        n, hp = c // 2, c % 2
        ch = slice(hp * P, (hp + 1) * P)
        wqkv = np.empty((P, 6, P), np.float32)
        for i, W in enumerate((Wk, Wq, Wv)):
            wt = np.asarray(W, np.float32)[ch].T  # (C, 128): [c_in, d_out]
            wqkv[:, 2 * i, :] = wt[0:P]
            wqkv[:, 2 * i + 1, :] = wt[P:C]
        wot = np.asarray(Wo, np.float32)[:, ch].T  # (128, 256)
        bvv = np.asarray(bv, np.float32)[ch]
        in_maps.append({
            "xx": xf[n],
            "wqkv": np.ascontiguousarray(wqkv.astype(bf16)),
            "wot": np.ascontiguousarray(wot.astype(bf16)),
            "wotb": np.ascontiguousarray(wot[P // 2:].astype(bf16)),
            "bq": np.ascontiguousarray(np.asarray(bq, np.float32)[ch].reshape(P, 1)),
            "bk": np.ascontiguousarray(np.asarray(bk, np.float32)[ch].reshape(P, 1)),
            "bv": np.ascontiguousarray(np.broadcast_to(bvv[None, :], (P, P))),
        })
    return in_maps


def run(inputs, **kwargs):
    """Run on 8 cores; returns (full output, BassKernelResults)."""
    nc = build_nc()
    in_maps = make_in_maps(**inputs)
    res = run_bass_kernel_spmd(nc, in_maps, core_ids=list(range(8)), **kwargs)
    x = np.asarray(inputs["x"], np.float32)
    bo = np.asarray(inputs["bo"], np.float32)
    N, _, H, W = x.shape
    out = np.empty((N, C, S), np.float32)
    for n in range(N):
        p0 = np.asarray(res.results[2 * n]["out"], np.float32).reshape(C, S)
        p1 = np.asarray(res.results[2 * n + 1]["out"], np.float32).reshape(C, S)
        out[n] = x[n].reshape(C, S) + p0 + p1 + bo[:, None]
    return out.reshape(N, C, H, W), res


def kernel(**inputs):
    out, _ = run(inputs)
    return out
